# revision 16
# baseline (speedup 1.0000x reference)
"""Single-head attention (B=4, N=4096, E=1024, H=64) on 8 TRN2 NeuronCores.

Sharding: core c = (batch b = c//2, query-half h = c%2). Each core computes the
full K/V projections for its batch and attention for its 2048 query rows.
Attention is permutation-invariant over keys, so each core receives its batch's
x pre-transposed ([E, N], embedding on partitions) with its OWN query half in
columns 0:2048 — the program is identical across cores (pure SPMD), only the
data differs.

Single unified pipeline per core (PSUM: 1 proj bank + 1 transpose bank +
2x2-bank S groups + 2 rotating O banks):
  - x streams in 2 MiB blocks (DMA-bound, ~47us); each block: k|v projections
    column-packed into one PSUM bank (tile_position (0,0)/(0,64)), q
    time-sharing the same bank; vT drains to high partitions and is
    PE-transposed into V-natural [128, 65] tiles with a fused ones column;
    kT/qT duplicated into partitions 64:128 (SBUF->SBUF DMA) so S^T matmuls
    run row-packed (alternating tile_position (0,0)/(64,0) by chunk parity).
  - attention for query blocks 0/1 rides chunk availability during the x
    stream; blocks 2/3 follow immediately after, same pools, no phase barrier.
  - per group of 2 key chunks: S^T = kT.T @ qT into a 2-bank PSUM tile,
    one exp((q.k)/8) pass on ScalarE -> bf16 P, then PV accumulates
    O[65, 512] += [V|1].T @ P (PV lags one group so PE never head-of-line
    blocks on ScalarE; row 64 of O = softmax denominators).
  - normalize: DMA row sums to DRAM, broadcast-DMA across 64 partitions,
    out = O[0:64] * approx(1/sums) -> outT [64, 2048] fp32.
Host assembles out[b, half] = outT.T.
"""

import tempfile

import numpy as np

import concourse.bass as bass
import concourse.tile as tile
from concourse import bacc, mybir
from concourse.bass_utils import run_bass_kernel_spmd
from concourse.masks import make_identity

B, N, E, H = 4, 4096, 1024, 64
NCORES = 8
NQ = N // 2  # query rows per core
QB = 512  # query block (free dim of attention matmuls)
NKC = N // 128  # 32 key chunks of 128
ECH = E // 128  # 8 embedding chunks of 128
NB = N // QB  # 8 projection column blocks
QBLKS = NQ // QB  # 4 query blocks per core
GRP = 2  # key chunks per S/exp group (PSUM banks per S tile)

F32 = mybir.dt.float32
BF16 = mybir.dt.bfloat16

SCALE = 1.0 / np.sqrt(H)


def build_kernel():
    nc = bacc.Bacc("TRN2", target_bir_lowering=False, debug=False, num_devices=NCORES)

    xT_d = nc.dram_tensor("xT", [E, N], F32, kind="ExternalInput")
    wT_d = nc.dram_tensor("wT", [E, 3 * H], F32, kind="ExternalInput")
    outT_d = nc.dram_tensor("outT", [H, NQ], F32, kind="ExternalOutput")
    sums_d = nc.dram_tensor("sums_bounce", [QBLKS, QB], F32)

    xT = xT_d.ap().rearrange("(c p) n -> p c n", p=128)  # [128, ECH, N]
    wT = wT_d.ap().rearrange("(c p) h -> p c h", p=128)  # [128, ECH, 192]
    outT = outT_d.ap()
    sums = sums_d.ap()

    with tile.TileContext(nc) as tc:
        with (
            tc.tile_pool(name="singles", bufs=1) as singles,
            tc.tile_pool(name="xpool", bufs=3) as xpool,
            tc.tile_pool(name="xbfpool", bufs=3) as xbfpool,
            tc.tile_pool(name="qkv", bufs=1) as qkv,
            tc.tile_pool(name="vstage", bufs=2) as vstage,
            tc.tile_pool(name="ppool", bufs=5) as ppool,
            tc.tile_pool(name="npool", bufs=2) as npool,
            tc.tile_pool(name="kvq_ps", bufs=1, space="PSUM") as kvq_pool,
            tc.tile_pool(name="tr_ps", bufs=1, space="PSUM") as tr_pool,
            tc.tile_pool(name="s_ps", bufs=2, space="PSUM") as s_pool,
            tc.tile_pool(name="o_ps", bufs=2, space="PSUM") as o_pool,
        ):
            # --- constants; wT first (it gates the first projection matmuls),
            # then x block 0 in quarter-pieces so casting starts early ---
            wT_f32 = singles.tile([128, ECH, 3 * H], F32)
            nc.sync.dma_start(out=wT_f32[:], in_=wT)
            x_t0 = xpool.tile([128, ECH, QB], F32, name="x_t")
            for piece in range(4):
                nc.sync.dma_start(
                    out=x_t0[:, 2 * piece : 2 * piece + 2, :],
                    in_=xT[:, 2 * piece : 2 * piece + 2, 0:QB],
                )
            wT_sb = singles.tile([128, ECH, 3 * H], BF16)
            nc.vector.tensor_copy(wT_sb[:], wT_f32[:])
            # identity for PE transposes, in the high partition half
            ident = singles.tile([128, H], BF16)
            make_identity(nc, ident[0:H, :])
            nc.scalar.dma_start(out=ident[H : 2 * H, :], in_=ident[0:H, :])
            # note: transposes with hi-half inputs use ident[64:128] (diagonal
            # duplicated there), lo-half inputs use ident[0:64]

            # persistent activations; rows 0:64 from projection drains,
            # rows 64:128 DMA duplicates enabling row-packed S^T matmuls
            kT_sb = qkv.tile([128, N], BF16)
            qT_sb = qkv.tile([128, NQ], BF16)
            v_all = qkv.tile([128, NKC, H + 1], BF16)
            nc.vector.memset(v_all[:, :, H : H + 1], 1.0)

            o_acc = [None] * QBLKS
            next_chunk = [0] * QBLKS
            pv_lag = [None] * QBLKS

            def s_matmul(s_slice, c, qsl):
                # k of even x-blocks lives in partitions 0:64, odd in 64:128
                lo = (c // 4) % 2 == 0
                r = slice(0, H) if lo else slice(H, 2 * H)
                nc.tensor.matmul(
                    s_slice,
                    kT_sb[r, c * 128 : (c + 1) * 128],
                    qT_sb[r, qsl],
                    start=True, stop=True,
                    tile_position=(0 if lo else H, 0),
                )

            def group_chunks(i):
                # pair one even-block (lo) chunk with one odd-block (hi) chunk
                lo = 8 * (i // 4) + (i % 4)
                return [lo, lo + 4]

            NGROUPS = NKC // GRP

            def emit_pv(qb, chunks, p_t):
                for j, c in enumerate(chunks):
                    nc.tensor.matmul(
                        o_acc[qb][:],
                        v_all[:, c, :],
                        p_t[:, j * QB : (j + 1) * QB],
                        start=(c == 0), stop=(c == NKC - 1),
                    )

            def emit_group(qb, i):
                if o_acc[qb] is None:
                    o_acc[qb] = o_pool.tile(
                        [H + 1, QB], F32, name=f"o_qb{qb}", tag="o_acc"
                    )
                qsl = slice(qb * QB, (qb + 1) * QB)
                chunks = group_chunks(i)
                s_t = s_pool.tile([128, GRP * QB], F32, name="s_t")
                for j, c in enumerate(chunks):
                    s_matmul(s_t[:, j * QB : (j + 1) * QB], c, qsl)
                p_t = ppool.tile([128, GRP * QB], BF16, name="p_t")
                nc.scalar.activation(
                    p_t[:], s_t[:],
                    mybir.ActivationFunctionType.Exp, scale=SCALE,
                )
                if pv_lag[qb] is not None:
                    emit_pv(qb, *pv_lag[qb])
                pv_lag[qb] = (chunks, p_t)

            def finish_qb(qb):
                emit_pv(qb, *pv_lag[qb])
                pv_lag[qb] = None
                o_t = o_acc[qb]
                s_row = npool.tile([1, QB], F32, name="s_row")
                nc.vector.tensor_copy(s_row[:], o_t[H : H + 1, :])
                nc.scalar.dma_start(out=sums[qb : qb + 1, :], in_=s_row[:])
                s_rep = npool.tile([H, QB], F32, name="s_rep")
                nc.scalar.dma_start(
                    out=s_rep[:],
                    in_=bass.AP(
                        tensor=sums.tensor, offset=qb * QB, ap=[[0, H], [1, QB]]
                    ),
                )
                r_rep = npool.tile([H, QB], F32, name="r_rep")
                nc.vector.reciprocal_approx_fast(out=r_rep[:], in_=s_rep[:])
                o_n = npool.tile([H, QB], F32, name="o_n")
                nc.vector.tensor_mul(o_n[:], o_t[0:H, :], r_rep[:])
                nc.sync.dma_start(
                    out=outT[:, qb * QB : (qb + 1) * QB], in_=o_n[:]
                )

            def emit_available(nb, qbs):
                avail_pairs = 4 * ((nb + 1) // 2)
                progress = True
                while progress:
                    progress = False
                    for qb in qbs:
                        if qb == 1 and nb < 1:
                            continue
                        if next_chunk[qb] < avail_pairs:
                            emit_group(qb, next_chunk[qb])
                            next_chunk[qb] += 1
                            progress = True

            # --- production loop: x stream -> projections + qb0/qb1 groups ---
            for nb in range(NB):
                if nb == 0:
                    x_t = x_t0
                else:
                    x_t = xpool.tile([128, ECH, QB], F32, name="x_t")
                    nc.sync.dma_start(
                        out=x_t[:], in_=xT[:, :, nb * QB : (nb + 1) * QB]
                    )
                x_bf = xbfpool.tile([128, ECH, QB], BF16)
                # cast engine split: DVE mostly, ACT for two mid blocks
                if nb == 0:
                    for piece in range(4):
                        psl = slice(2 * piece, 2 * piece + 2)
                        nc.vector.tensor_copy(x_bf[:, psl, :], x_t[:, psl, :])
                elif nb in (3, 5):
                    nc.scalar.copy(x_bf[:], x_t[:])
                else:
                    nc.vector.tensor_copy(x_bf[:, 0:4, :], x_t[:, 0:4, :])
                    nc.vector.tensor_copy(x_bf[:, 4:8, :], x_t[:, 4:8, :])
                want_q = nb < QBLKS
                kv_ps = kvq_pool.tile([128, QB], F32, name="kv_ps", tag="kvq")
                for ec in range(ECH):
                    rhs = x_bf[:, ec, :]
                    first, last = ec == 0, ec == ECH - 1
                    nc.tensor.matmul(
                        kv_ps[0:H, :], wT_sb[:, ec, 0:H], rhs,
                        start=first, stop=last, tile_position=(0, 0),
                    )
                    nc.tensor.matmul(
                        kv_ps[H:128, :], wT_sb[:, ec, 2 * H : 3 * H], rhs,
                        start=first, stop=last, tile_position=(0, H),
                    )
                nsl = slice(nb * QB, (nb + 1) * QB)
                nc.vector.tensor_copy(kT_sb[0:H, nsl], kv_ps[0:H, :])
                nc.gpsimd.dma_start(out=kT_sb[H:128, nsl], in_=kT_sb[0:H, nsl])
                # vT (hi partitions) -> bf16 staging -> PE transpose -> V tiles
                vT_blk = vstage.tile([128, QB], BF16)
                nc.vector.tensor_copy(vT_blk[H:128, :], kv_ps[H:128, :])
                if want_q:
                    # q time-shares the kv bank (same PE column group as k)
                    q_ps = kvq_pool.tile([H, QB], F32, name="q_ps", tag="kvq")
                    for ec in range(ECH):
                        nc.tensor.matmul(
                            q_ps[:], wT_sb[:, ec, H : 2 * H], x_bf[:, ec, :],
                            start=(ec == 0), stop=(ec == ECH - 1),
                        )
                    nc.vector.tensor_copy(qT_sb[0:H, nsl], q_ps[:])
                    nc.gpsimd.dma_start(
                        out=qT_sb[H:128, nsl], in_=qT_sb[0:H, nsl]
                    )
                for j in range(QB // 128):
                    c = nb * (QB // 128) + j
                    v_tr = tr_pool.tile([128, H], BF16)
                    nc.tensor.transpose(
                        v_tr[:],
                        vT_blk[H:128, j * 128 : (j + 1) * 128],
                        ident[H : 2 * H, :],
                        tile_position=(H, 0),
                    )
                    nc.vector.tensor_copy(v_all[:, c, 0:H], v_tr[:])
                emit_available(nb, (0, 1))

            # --- drain: qb0/qb1 finish, qb2/qb3 flow through the same pools ---
            finish_qb(0)
            finish_qb(1)
            for qb in (2, 3):
                while next_chunk[qb] < NGROUPS:
                    emit_group(qb, next_chunk[qb])
                    next_chunk[qb] += 1
                finish_qb(qb)

    nc.compile()
    return nc


_NC_CACHE = {}


def _get_nc():
    if "nc" not in _NC_CACHE:
        _NC_CACHE["nc"] = build_kernel()
    return _NC_CACHE["nc"]


def _make_in_maps(x, Wk, Wq, Wv):
    wT = np.ascontiguousarray(
        np.concatenate([Wk.T, Wq.T, Wv.T], axis=1), dtype=np.float32
    )
    in_maps = []
    for c in range(NCORES):
        b, h = divmod(c, 2)
        xb = np.asarray(x[b], dtype=np.float32)
        if h == 1:
            xb = np.concatenate([xb[NQ:], xb[:NQ]], axis=0)
        in_maps.append({"xT": np.ascontiguousarray(xb.T), "wT": wT})
    return in_maps


def kernel(x, Wk, Wq, Wv, _trace=False, _tmpdir=None):
    nc = _get_nc()
    in_maps = _make_in_maps(x, Wk, Wq, Wv)
    kwargs = {}
    if _trace:
        kwargs = dict(trace=True, tmpdir=_tmpdir or tempfile.mkdtemp())
    res = run_bass_kernel_spmd(nc, in_maps, core_ids=list(range(NCORES)), **kwargs)
    out = np.empty((B, N, H), np.float32)
    for c in range(NCORES):
        b, h = divmod(c, 2)
        out[b, h * NQ : (h + 1) * NQ, :] = res.results[c]["outT"].T
    if _trace:
        return out, res
    return out


# revision 17
# speedup vs baseline: 1.0457x; 1.0457x over previous
"""Single-head attention (B=4, N=4096, E=1024, H=64) on 8 TRN2 NeuronCores.

Sharding: core c = (batch b = c//2, query-half h = c%2). Each core computes the
full K/V projections for its batch and attention for its 2048 query rows.
Attention is permutation-invariant over keys, so each core receives its batch's
x pre-transposed ([E, N], embedding on partitions) with its OWN query half in
columns 0:2048 — the program is identical across cores (pure SPMD), only the
data differs.

Single unified pipeline per core (PSUM: 1 proj bank + 1 transpose bank +
2x2-bank S groups + 2 rotating O banks):
  - x streams in 2 MiB blocks (DMA-bound, ~47us); each block: k|v projections
    column-packed into one PSUM bank (tile_position (0,0)/(0,64)), q
    time-sharing the same bank; vT drains to high partitions and is
    PE-transposed into V-natural [128, 65] tiles with a fused ones column;
    kT/qT duplicated into partitions 64:128 (SBUF->SBUF DMA) so S^T matmuls
    run row-packed (alternating tile_position (0,0)/(64,0) by chunk parity).
  - attention for query blocks 0/1 rides chunk availability during the x
    stream; blocks 2/3 follow immediately after, same pools, no phase barrier.
  - per group of 2 key chunks: S^T = kT.T @ qT into a 2-bank PSUM tile,
    one exp((q.k)/8) pass on ScalarE -> bf16 P, then PV accumulates
    O[65, 512] += [V|1].T @ P (PV lags one group so PE never head-of-line
    blocks on ScalarE; row 64 of O = softmax denominators).
  - normalize: DMA row sums to DRAM, broadcast-DMA across 64 partitions,
    out = O[0:64] * approx(1/sums) -> outT [64, 2048] fp32.
Host assembles out[b, half] = outT.T.
"""

import tempfile

import numpy as np

import concourse.bass as bass
import concourse.tile as tile
from concourse import bacc, mybir
from concourse.bass_utils import run_bass_kernel_spmd
from concourse.masks import make_identity

B, N, E, H = 4, 4096, 1024, 64
NCORES = 8
NQ = N // 2  # query rows per core
QB = 512  # query block (free dim of attention matmuls)
NKC = N // 128  # 32 key chunks of 128
ECH = E // 128  # 8 embedding chunks of 128
NB = N // QB  # 8 projection column blocks
QBLKS = NQ // QB  # 4 query blocks per core
GRP = 2  # key chunks per S/exp group (PSUM banks per S tile)

F32 = mybir.dt.float32
BF16 = mybir.dt.bfloat16

SCALE = 1.0 / np.sqrt(H)


def build_kernel():
    nc = bacc.Bacc("TRN2", target_bir_lowering=False, debug=False, num_devices=NCORES)

    xT_d = nc.dram_tensor("xT", [E, N], F32, kind="ExternalInput")
    wT_d = nc.dram_tensor("wT", [E, 3 * H], F32, kind="ExternalInput")
    outT_d = nc.dram_tensor("outT", [H, NQ], F32, kind="ExternalOutput")
    sums_d = nc.dram_tensor("sums_bounce", [QBLKS, QB], F32)

    xT = xT_d.ap().rearrange("(c p) n -> p c n", p=128)  # [128, ECH, N]
    wT = wT_d.ap().rearrange("(c p) h -> p c h", p=128)  # [128, ECH, 192]
    outT = outT_d.ap()
    sums = sums_d.ap()

    with tile.TileContext(nc) as tc:
        with (
            tc.tile_pool(name="singles", bufs=1) as singles,
            tc.tile_pool(name="xpool", bufs=3) as xpool,
            tc.tile_pool(name="xbfpool", bufs=3) as xbfpool,
            tc.tile_pool(name="qkv", bufs=1) as qkv,
            tc.tile_pool(name="vstage", bufs=2) as vstage,
            tc.tile_pool(name="ppool", bufs=5) as ppool,
            tc.tile_pool(name="npool", bufs=2) as npool,
            tc.tile_pool(name="kvq_ps", bufs=1, space="PSUM") as kvq_pool,
            tc.tile_pool(name="tr_ps", bufs=1, space="PSUM") as tr_pool,
            tc.tile_pool(name="s_ps", bufs=2, space="PSUM") as s_pool,
            tc.tile_pool(name="o_ps", bufs=2, space="PSUM") as o_pool,
        ):
            # --- constants; wT first (it gates the first projection matmuls),
            # then x block 0 in quarter-pieces so casting starts early ---
            wT_f32 = singles.tile([128, ECH, 3 * H], F32)
            nc.sync.dma_start(out=wT_f32[:], in_=wT)
            x_t0 = xpool.tile([128, ECH, QB], F32, name="x_t")
            for piece in range(4):
                nc.sync.dma_start(
                    out=x_t0[:, 2 * piece : 2 * piece + 2, :],
                    in_=xT[:, 2 * piece : 2 * piece + 2, 0:QB],
                )
            wT_sb = singles.tile([128, ECH, 3 * H], BF16)
            nc.vector.tensor_copy(wT_sb[:], wT_f32[:])
            # identity for PE transposes, in the high partition half
            ident = singles.tile([128, H], BF16)
            make_identity(nc, ident[0:H, :])
            nc.scalar.dma_start(out=ident[H : 2 * H, :], in_=ident[0:H, :])
            # note: transposes with hi-half inputs use ident[64:128] (diagonal
            # duplicated there), lo-half inputs use ident[0:64]

            # persistent activations; rows 0:64 from projection drains,
            # rows 64:128 DMA duplicates enabling row-packed S^T matmuls
            kT_sb = qkv.tile([128, N], BF16)
            qT_sb = qkv.tile([128, NQ], BF16)
            v_all = qkv.tile([128, NKC, H + 1], BF16)
            nc.vector.memset(v_all[:, :, H : H + 1], 1.0)

            # PE warmup: ~4us of junk matmuls on the weights so the HAM
            # clock-gate opens before the first real projection matmuls
            warm_ps = kvq_pool.tile([128, QB], F32, name="warm_ps", tag="kvq")
            for w in range(40):
                nc.tensor.matmul(
                    warm_ps[0:H, 0:192], wT_sb[:, 0, 0:H], wT_sb[:, 1, :],
                    start=True, stop=True, tile_position=(0, 0),
                )

            o_acc = [None] * QBLKS
            next_chunk = [0] * QBLKS
            pv_lag = [None] * QBLKS

            def s_matmul(s_slice, c, qsl):
                # k of even x-blocks lives in partitions 0:64, odd in 64:128
                lo = (c // 4) % 2 == 0
                r = slice(0, H) if lo else slice(H, 2 * H)
                nc.tensor.matmul(
                    s_slice,
                    kT_sb[r, c * 128 : (c + 1) * 128],
                    qT_sb[r, qsl],
                    start=True, stop=True,
                    tile_position=(0 if lo else H, 0),
                )

            def group_chunks(i):
                # pair one even-block (lo) chunk with one odd-block (hi) chunk
                lo = 8 * (i // 4) + (i % 4)
                return [lo, lo + 4]

            NGROUPS = NKC // GRP

            def emit_pv(qb, chunks, p_t):
                for j, c in enumerate(chunks):
                    nc.tensor.matmul(
                        o_acc[qb][:],
                        v_all[:, c, :],
                        p_t[:, j * QB : (j + 1) * QB],
                        start=(c == 0), stop=(c == NKC - 1),
                    )

            def emit_group(qb, i):
                if o_acc[qb] is None:
                    o_acc[qb] = o_pool.tile(
                        [H + 1, QB], F32, name=f"o_qb{qb}", tag="o_acc"
                    )
                qsl = slice(qb * QB, (qb + 1) * QB)
                chunks = group_chunks(i)
                s_t = s_pool.tile([128, GRP * QB], F32, name="s_t")
                for j, c in enumerate(chunks):
                    s_matmul(s_t[:, j * QB : (j + 1) * QB], c, qsl)
                p_t = ppool.tile([128, GRP * QB], BF16, name="p_t")
                nc.scalar.activation(
                    p_t[:], s_t[:],
                    mybir.ActivationFunctionType.Exp, scale=SCALE,
                )
                if pv_lag[qb] is not None:
                    emit_pv(qb, *pv_lag[qb])
                pv_lag[qb] = (chunks, p_t)

            def finish_qb(qb):
                emit_pv(qb, *pv_lag[qb])
                pv_lag[qb] = None
                o_t = o_acc[qb]
                s_row = npool.tile([1, QB], F32, name="s_row")
                nc.vector.tensor_copy(s_row[:], o_t[H : H + 1, :])
                nc.scalar.dma_start(out=sums[qb : qb + 1, :], in_=s_row[:])
                s_rep = npool.tile([H, QB], F32, name="s_rep")
                nc.scalar.dma_start(
                    out=s_rep[:],
                    in_=bass.AP(
                        tensor=sums.tensor, offset=qb * QB, ap=[[0, H], [1, QB]]
                    ),
                )
                r_rep = npool.tile([H, QB], F32, name="r_rep")
                nc.vector.reciprocal_approx_fast(out=r_rep[:], in_=s_rep[:])
                o_n = npool.tile([H, QB], F32, name="o_n")
                nc.vector.tensor_mul(o_n[:], o_t[0:H, :], r_rep[:])
                nc.sync.dma_start(
                    out=outT[:, qb * QB : (qb + 1) * QB], in_=o_n[:]
                )

            def emit_available(nb, qbs):
                avail_pairs = 4 * ((nb + 1) // 2)
                progress = True
                while progress:
                    progress = False
                    for qb in qbs:
                        if qb == 1 and nb < 1:
                            continue
                        if next_chunk[qb] < avail_pairs:
                            emit_group(qb, next_chunk[qb])
                            next_chunk[qb] += 1
                            progress = True

            # --- production loop: x stream -> projections + qb0/qb1 groups;
            # DMA + bf16 cast run two blocks ahead of the projection matmuls
            # so the PE is never paced by the cast chain ---
            x_bf_tiles = {}

            def load_block(nb):
                if nb == 0:
                    x_t = x_t0
                else:
                    x_t = xpool.tile([128, ECH, QB], F32, name="x_t")
                    nc.sync.dma_start(
                        out=x_t[:], in_=xT[:, :, nb * QB : (nb + 1) * QB]
                    )
                x_bf = xbfpool.tile([128, ECH, QB], BF16)
                if nb == 0:
                    for piece in range(4):
                        psl = slice(2 * piece, 2 * piece + 2)
                        nc.vector.tensor_copy(x_bf[:, psl, :], x_t[:, psl, :])
                else:
                    nc.vector.tensor_copy(x_bf[:, 0:4, :], x_t[:, 0:4, :])
                    nc.vector.tensor_copy(x_bf[:, 4:8, :], x_t[:, 4:8, :])
                x_bf_tiles[nb] = x_bf

            load_block(0)
            load_block(1)
            for nb in range(NB):
                if nb + 2 < NB:
                    load_block(nb + 2)
                x_bf = x_bf_tiles.pop(nb)
                want_q = nb < QBLKS
                kv_ps = kvq_pool.tile([128, QB], F32, name="kv_ps", tag="kvq")
                for ec in range(ECH):
                    rhs = x_bf[:, ec, :]
                    first, last = ec == 0, ec == ECH - 1
                    nc.tensor.matmul(
                        kv_ps[0:H, :], wT_sb[:, ec, 0:H], rhs,
                        start=first, stop=last, tile_position=(0, 0),
                    )
                    nc.tensor.matmul(
                        kv_ps[H:128, :], wT_sb[:, ec, 2 * H : 3 * H], rhs,
                        start=first, stop=last, tile_position=(0, H),
                    )
                nsl = slice(nb * QB, (nb + 1) * QB)
                nc.vector.tensor_copy(kT_sb[0:H, nsl], kv_ps[0:H, :])
                nc.gpsimd.dma_start(out=kT_sb[H:128, nsl], in_=kT_sb[0:H, nsl])
                # vT (hi partitions) -> bf16 staging -> PE transpose -> V tiles
                vT_blk = vstage.tile([128, QB], BF16)
                nc.vector.tensor_copy(vT_blk[H:128, :], kv_ps[H:128, :])
                if want_q:
                    # q time-shares the kv bank (same PE column group as k)
                    q_ps = kvq_pool.tile([H, QB], F32, name="q_ps", tag="kvq")
                    for ec in range(ECH):
                        nc.tensor.matmul(
                            q_ps[:], wT_sb[:, ec, H : 2 * H], x_bf[:, ec, :],
                            start=(ec == 0), stop=(ec == ECH - 1),
                        )
                    nc.vector.tensor_copy(qT_sb[0:H, nsl], q_ps[:])
                    nc.gpsimd.dma_start(
                        out=qT_sb[H:128, nsl], in_=qT_sb[0:H, nsl]
                    )
                for j in range(QB // 128):
                    c = nb * (QB // 128) + j
                    v_tr = tr_pool.tile([128, H], BF16)
                    nc.tensor.transpose(
                        v_tr[:],
                        vT_blk[H:128, j * 128 : (j + 1) * 128],
                        ident[H : 2 * H, :],
                        tile_position=(H, 0),
                    )
                    nc.vector.tensor_copy(v_all[:, c, 0:H], v_tr[:])
                emit_available(nb, (0, 1))

            # --- drain: qb0/qb1 finish, qb2/qb3 flow through the same pools ---
            finish_qb(0)
            finish_qb(1)
            for qb in (2, 3):
                while next_chunk[qb] < NGROUPS:
                    emit_group(qb, next_chunk[qb])
                    next_chunk[qb] += 1
                finish_qb(qb)

    nc.compile()
    return nc


_NC_CACHE = {}


def _get_nc():
    if "nc" not in _NC_CACHE:
        _NC_CACHE["nc"] = build_kernel()
    return _NC_CACHE["nc"]


def _make_in_maps(x, Wk, Wq, Wv):
    wT = np.ascontiguousarray(
        np.concatenate([Wk.T, Wq.T, Wv.T], axis=1), dtype=np.float32
    )
    in_maps = []
    for c in range(NCORES):
        b, h = divmod(c, 2)
        xb = np.asarray(x[b], dtype=np.float32)
        if h == 1:
            xb = np.concatenate([xb[NQ:], xb[:NQ]], axis=0)
        in_maps.append({"xT": np.ascontiguousarray(xb.T), "wT": wT})
    return in_maps


def kernel(x, Wk, Wq, Wv, _trace=False, _tmpdir=None):
    nc = _get_nc()
    in_maps = _make_in_maps(x, Wk, Wq, Wv)
    kwargs = {}
    if _trace:
        kwargs = dict(trace=True, tmpdir=_tmpdir or tempfile.mkdtemp())
    res = run_bass_kernel_spmd(nc, in_maps, core_ids=list(range(NCORES)), **kwargs)
    out = np.empty((B, N, H), np.float32)
    for c in range(NCORES):
        b, h = divmod(c, 2)
        out[b, h * NQ : (h + 1) * NQ, :] = res.results[c]["outT"].T
    if _trace:
        return out, res
    return out
